# revision 2
# baseline (speedup 1.0000x reference)
"""v2b: baseline structure, tuned gather path.

Changes vs baseline kernel.py:
- bf16 node-projection tables (gather is descriptor-bound; bf16 halves
  DVE/ACT work and input-rebind cost)
- 4 SWDGE queues, gathers interleaved across them, single_packet=False
  (~2x descriptor throughput, measured)
- deeper tile pools for more gathers in flight
"""

import sys

for _p in ("/opt/trn_rl_repo",):
    if _p not in sys.path:
        sys.path.insert(0, _p)

import numpy as np

import concourse.bacc as bacc
import concourse.mybir as mybir
import concourse.tile as tile
from concourse.bass_utils import run_bass_kernel_spmd

H = 128
N_ING = 20000
N_CMP = 10000
N_EDGE = 1000000
NCORES = 8
E_CORE = N_EDGE // NCORES
G = 1024
NT = -(-E_CORE // G)
E_PAD = NT * G
NBLK = G // 128
NQ = 4

f32 = mybir.dt.float32
bf16 = mybir.dt.bfloat16
i16 = mybir.dt.int16
AF = mybir.ActivationFunctionType
ALU = mybir.AluOpType

_prog_cache = {}
_last_in_maps = None


def _build_program(n_pos):
    nc = bacc.Bacc("TRN2", target_bir_lowering=False, debug=False,
                   num_swdge_queues=NQ)
    a_ing = nc.dram_tensor("a_ing", [N_ING, H], bf16, kind="ExternalInput")
    a_cmp = nc.dram_tensor("a_cmp", [N_CMP, H], bf16, kind="ExternalInput")
    sidx = nc.dram_tensor("sidx", [NT, 128, G // 16], i16, kind="ExternalInput")
    didx = nc.dram_tensor("didx", [NT, 128, G // 16], i16, kind="ExternalInput")
    b2rep = nc.dram_tensor("b2rep", [128, 1], f32, kind="ExternalInput")
    outd = nc.dram_tensor("out", [NT, 128, NBLK], f32, kind="ExternalOutput")

    with tile.TileContext(nc) as tc:
        with (
            tc.tile_pool(name="const", bufs=1) as constp,
            tc.tile_pool(name="idx", bufs=8) as idxp,
            tc.tile_pool(name="gath", bufs=6) as gathp,
            tc.tile_pool(name="trash", bufs=2) as trashp,
            tc.tile_pool(name="acc", bufs=3) as accp,
        ):
            b2t = constp.tile([128, 1], f32)
            nc.sync.dma_start(out=b2t[:], in_=b2rep[:])

            for t in range(NT):
                st = idxp.tile([128, G // 16], i16, tag="sidx")
                nc.sync.dma_start(out=st[:], in_=sidx[t, :, :])
                dt_ = idxp.tile([128, G // 16], i16, tag="didx")
                nc.sync.dma_start(out=dt_[:], in_=didx[t, :, :])

                gs = gathp.tile([128, NBLK, H], bf16, tag="gs")
                nc.gpsimd.dma_gather(
                    out_ap=gs[:], in_ap=a_ing[:], idxs_ap=st[:],
                    num_idxs=G, num_idxs_reg=G, elem_size=H,
                    single_packet=False, queue_num=(2 * t) % NQ,
                )
                gd = gathp.tile([128, NBLK, H], bf16, tag="gd")
                nc.gpsimd.dma_gather(
                    out_ap=gd[:], in_ap=a_cmp[:], idxs_ap=dt_[:],
                    num_idxs=G, num_idxs_reg=G, elem_size=H,
                    single_packet=False, queue_num=(2 * t + 1) % NQ,
                )

                # u = A_ing[src] + A_cmp[dst]  (b1, |w2| folded in)
                nc.vector.tensor_tensor(out=gs[:], in0=gs[:], in1=gd[:], op=ALU.add)

                pos = accp.tile([128, NBLK], f32, tag="pos")
                neg = accp.tile([128, NBLK], f32, tag="neg")
                trash = trashp.tile([128, H], f32, tag="trash")
                for b in range(NBLK):
                    if n_pos > 0:
                        nc.scalar.activation(
                            trash[:, :n_pos], gs[:, b, :n_pos], AF.Relu,
                            accum_out=pos[:, b:b + 1],
                        )
                    if n_pos < H:
                        nc.scalar.activation(
                            trash[:, :H - n_pos], gs[:, b, n_pos:], AF.Relu,
                            accum_out=neg[:, b:b + 1],
                        )

                outv = accp.tile([128, NBLK], f32, tag="outv")
                if 0 < n_pos < H:
                    logit = accp.tile([128, NBLK], f32, tag="logit")
                    nc.vector.tensor_tensor(
                        out=logit[:], in0=pos[:], in1=neg[:], op=ALU.subtract
                    )
                    nc.scalar.activation(outv[:], logit[:], AF.Sigmoid,
                                         bias=b2t[:, 0:1])
                elif n_pos == H:
                    nc.scalar.activation(outv[:], pos[:], AF.Sigmoid,
                                         bias=b2t[:, 0:1])
                else:
                    nc.scalar.activation(outv[:], neg[:], AF.Sigmoid,
                                         bias=b2t[:, 0:1], scale=-1.0)
                nc.sync.dma_start(out=outd[t, :, :], in_=outv[:])

    nc.compile()
    return nc


def _wrap_idx(ids: np.ndarray) -> np.ndarray:
    w = ids.reshape(NT, G // 16, 16).transpose(0, 2, 1)
    return np.ascontiguousarray(np.tile(w, (1, 8, 1)), dtype=np.int16)


def _to_bf16(a):
    import jax.numpy as jnp
    return np.asarray(jnp.asarray(a, dtype=jnp.bfloat16))


def kernel(x_ingredient, x_compound, edge_index, W1, b1, W2, b2):
    x_ing = np.asarray(x_ingredient, dtype=np.float32)
    x_cmp = np.asarray(x_compound, dtype=np.float32)
    W1 = np.asarray(W1, dtype=np.float32)
    b1 = np.asarray(b1, dtype=np.float32)
    W2 = np.asarray(W2, dtype=np.float32).reshape(H)
    b2 = np.asarray(b2, dtype=np.float32)
    src = np.asarray(edge_index[0]).astype(np.int64)
    dst = np.asarray(edge_index[1]).astype(np.int64)

    pos_mask = W2 >= 0
    perm = np.concatenate([np.nonzero(pos_mask)[0], np.nonzero(~pos_mask)[0]])
    n_pos = int(pos_mask.sum())
    w2abs = np.abs(W2[perm])
    W1p = W1[:, perm] * w2abs
    b1p = b1[perm] * w2abs

    a_ing = _to_bf16(x_ing @ W1p[:H])
    a_cmp = _to_bf16(x_cmp @ W1p[H:] + b1p)
    b2rep = np.full((128, 1), float(b2.reshape(-1)[0]), dtype=np.float32)

    if n_pos not in _prog_cache:
        _prog_cache[n_pos] = _build_program(n_pos)
    nc = _prog_cache[n_pos]
    _prog_cache["prog"] = nc

    in_maps = []
    for c in range(NCORES):
        s = np.zeros(E_PAD, dtype=np.int64)
        d = np.zeros(E_PAD, dtype=np.int64)
        s[:E_CORE] = src[c * E_CORE:(c + 1) * E_CORE]
        d[:E_CORE] = dst[c * E_CORE:(c + 1) * E_CORE]
        in_maps.append({
            "a_ing": a_ing,
            "a_cmp": a_cmp,
            "sidx": _wrap_idx(s),
            "didx": _wrap_idx(d),
            "b2rep": b2rep,
        })

    global _last_in_maps
    _last_in_maps = in_maps
    res = run_bass_kernel_spmd(nc, in_maps, list(range(NCORES)))

    outs = []
    for c in range(NCORES):
        o = res.results[c]["out"]
        outs.append(o.transpose(0, 2, 1).reshape(E_PAD)[:E_CORE])
    return np.concatenate(outs).reshape(N_EDGE, 1).astype(np.float32)





# revision 3
# speedup vs baseline: 1.0956x; 1.0956x over previous
"""v2b: baseline structure, tuned gather path.

Changes vs baseline kernel.py:
- bf16 node-projection tables (gather is descriptor-bound; bf16 halves
  DVE/ACT work and input-rebind cost)
- 4 SWDGE queues, gathers interleaved across them, single_packet=False
  (~2x descriptor throughput, measured)
- deep tile pools (10 gather / 12 idx buffers) keep ~5 tiles of
  gathers in flight across the 4 queues
"""

import sys

for _p in ("/opt/trn_rl_repo",):
    if _p not in sys.path:
        sys.path.insert(0, _p)

import numpy as np

import concourse.bacc as bacc
import concourse.mybir as mybir
import concourse.tile as tile
from concourse.bass_utils import run_bass_kernel_spmd

H = 128
N_ING = 20000
N_CMP = 10000
N_EDGE = 1000000
NCORES = 8
E_CORE = N_EDGE // NCORES
G = 1024
NT = -(-E_CORE // G)
E_PAD = NT * G
NBLK = G // 128
NQ = 4

f32 = mybir.dt.float32
bf16 = mybir.dt.bfloat16
i16 = mybir.dt.int16
AF = mybir.ActivationFunctionType
ALU = mybir.AluOpType

_prog_cache = {}
_last_in_maps = None


def _build_program(n_pos):
    nc = bacc.Bacc("TRN2", target_bir_lowering=False, debug=False,
                   num_swdge_queues=NQ)
    a_ing = nc.dram_tensor("a_ing", [N_ING, H], bf16, kind="ExternalInput")
    a_cmp = nc.dram_tensor("a_cmp", [N_CMP, H], bf16, kind="ExternalInput")
    sidx = nc.dram_tensor("sidx", [NT, 128, G // 16], i16, kind="ExternalInput")
    didx = nc.dram_tensor("didx", [NT, 128, G // 16], i16, kind="ExternalInput")
    b2rep = nc.dram_tensor("b2rep", [128, 1], f32, kind="ExternalInput")
    outd = nc.dram_tensor("out", [NT, 128, NBLK], f32, kind="ExternalOutput")

    with tile.TileContext(nc) as tc:
        with (
            tc.tile_pool(name="const", bufs=1) as constp,
            tc.tile_pool(name="idx", bufs=12) as idxp,
            tc.tile_pool(name="gath", bufs=10) as gathp,
            tc.tile_pool(name="trash", bufs=2) as trashp,
            tc.tile_pool(name="acc", bufs=3) as accp,
        ):
            b2t = constp.tile([128, 1], f32)
            nc.sync.dma_start(out=b2t[:], in_=b2rep[:])

            for t in range(NT):
                st = idxp.tile([128, G // 16], i16, tag="sidx")
                nc.sync.dma_start(out=st[:], in_=sidx[t, :, :])
                dt_ = idxp.tile([128, G // 16], i16, tag="didx")
                nc.sync.dma_start(out=dt_[:], in_=didx[t, :, :])

                gs = gathp.tile([128, NBLK, H], bf16, tag="gs")
                nc.gpsimd.dma_gather(
                    out_ap=gs[:], in_ap=a_ing[:], idxs_ap=st[:],
                    num_idxs=G, num_idxs_reg=G, elem_size=H,
                    single_packet=False, queue_num=(2 * t) % NQ,
                )
                gd = gathp.tile([128, NBLK, H], bf16, tag="gd")
                nc.gpsimd.dma_gather(
                    out_ap=gd[:], in_ap=a_cmp[:], idxs_ap=dt_[:],
                    num_idxs=G, num_idxs_reg=G, elem_size=H,
                    single_packet=False, queue_num=(2 * t + 1) % NQ,
                )

                # u = A_ing[src] + A_cmp[dst]  (b1, |w2| folded in)
                nc.vector.tensor_tensor(out=gs[:], in0=gs[:], in1=gd[:], op=ALU.add)

                pos = accp.tile([128, NBLK], f32, tag="pos")
                neg = accp.tile([128, NBLK], f32, tag="neg")
                trash = trashp.tile([128, H], f32, tag="trash")
                for b in range(NBLK):
                    if n_pos > 0:
                        nc.scalar.activation(
                            trash[:, :n_pos], gs[:, b, :n_pos], AF.Relu,
                            accum_out=pos[:, b:b + 1],
                        )
                    if n_pos < H:
                        nc.scalar.activation(
                            trash[:, :H - n_pos], gs[:, b, n_pos:], AF.Relu,
                            accum_out=neg[:, b:b + 1],
                        )

                outv = accp.tile([128, NBLK], f32, tag="outv")
                if 0 < n_pos < H:
                    logit = accp.tile([128, NBLK], f32, tag="logit")
                    nc.vector.tensor_tensor(
                        out=logit[:], in0=pos[:], in1=neg[:], op=ALU.subtract
                    )
                    nc.scalar.activation(outv[:], logit[:], AF.Sigmoid,
                                         bias=b2t[:, 0:1])
                elif n_pos == H:
                    nc.scalar.activation(outv[:], pos[:], AF.Sigmoid,
                                         bias=b2t[:, 0:1])
                else:
                    nc.scalar.activation(outv[:], neg[:], AF.Sigmoid,
                                         bias=b2t[:, 0:1], scale=-1.0)
                nc.sync.dma_start(out=outd[t, :, :], in_=outv[:])

    nc.compile()
    return nc


def _wrap_idx(ids: np.ndarray) -> np.ndarray:
    w = ids.reshape(NT, G // 16, 16).transpose(0, 2, 1)
    return np.ascontiguousarray(np.tile(w, (1, 8, 1)), dtype=np.int16)


def _to_bf16(a):
    import jax.numpy as jnp
    return np.asarray(jnp.asarray(a, dtype=jnp.bfloat16))


def kernel(x_ingredient, x_compound, edge_index, W1, b1, W2, b2):
    x_ing = np.asarray(x_ingredient, dtype=np.float32)
    x_cmp = np.asarray(x_compound, dtype=np.float32)
    W1 = np.asarray(W1, dtype=np.float32)
    b1 = np.asarray(b1, dtype=np.float32)
    W2 = np.asarray(W2, dtype=np.float32).reshape(H)
    b2 = np.asarray(b2, dtype=np.float32)
    src = np.asarray(edge_index[0]).astype(np.int64)
    dst = np.asarray(edge_index[1]).astype(np.int64)

    pos_mask = W2 >= 0
    perm = np.concatenate([np.nonzero(pos_mask)[0], np.nonzero(~pos_mask)[0]])
    n_pos = int(pos_mask.sum())
    w2abs = np.abs(W2[perm])
    W1p = W1[:, perm] * w2abs
    b1p = b1[perm] * w2abs

    a_ing = _to_bf16(x_ing @ W1p[:H])
    a_cmp = _to_bf16(x_cmp @ W1p[H:] + b1p)
    b2rep = np.full((128, 1), float(b2.reshape(-1)[0]), dtype=np.float32)

    if n_pos not in _prog_cache:
        _prog_cache[n_pos] = _build_program(n_pos)
    nc = _prog_cache[n_pos]
    _prog_cache["prog"] = nc

    in_maps = []
    for c in range(NCORES):
        s = np.zeros(E_PAD, dtype=np.int64)
        d = np.zeros(E_PAD, dtype=np.int64)
        s[:E_CORE] = src[c * E_CORE:(c + 1) * E_CORE]
        d[:E_CORE] = dst[c * E_CORE:(c + 1) * E_CORE]
        in_maps.append({
            "a_ing": a_ing,
            "a_cmp": a_cmp,
            "sidx": _wrap_idx(s),
            "didx": _wrap_idx(d),
            "b2rep": b2rep,
        })

    global _last_in_maps
    _last_in_maps = in_maps
    res = run_bass_kernel_spmd(nc, in_maps, list(range(NCORES)))

    outs = []
    for c in range(NCORES):
        o = res.results[c]["out"]
        outs.append(o.transpose(0, 2, 1).reshape(E_PAD)[:E_CORE])
    return np.concatenate(outs).reshape(N_EDGE, 1).astype(np.float32)



